# revision 1
# baseline (speedup 1.0000x reference)
"""SE(3) compose-scan Trainium2 kernel (nn_ComposeRt).

x [131072, 32, 3, 4] fp32 -> cumulative compose along axis 1:
out[b,0] = x[b,0]; out[b,n] = out[b,n-1] o x[b,n],
[rA|tA] o [rB|tB] = [rA@rB | tA + rA@tB].

Sharding: pure data parallel over batch across 8 NeuronCores.
Per core: batch b_local = t*(P*F) + p*F + f (mega-tile t, partition p,
slot f). DRAM I/O blocks [MEGA*HALVES, P, F*NSUB*12]; block (t, h) holds
n-range [h*NSUB, (h+1)*NSUB), SBUF layout [p][f][n][i*4+j].

Variants:
- "dve": per scan step, six vector-engine tensor ops (3 broadcast
  multiplies, 2 accumulate adds, translation add) batched over (f, i, j).
- "cumsum": the scalar engine materializes both operands of all nine
  rotation products as contiguous per-partition streams (A replicated
  over j, B replicated over i); one custom DVE op computes the running
  sum of products over the stream; a strided subtract of group
  boundaries extracts the nine dot products; a small add applies the
  carried translation. 51 instead of 63 DVE element-cycles per compose
  and 3 instead of 6 DVE instructions per step.
"""

import sys

if "/opt/trn_rl_repo" not in sys.path:
    sys.path.insert(0, "/opt/trn_rl_repo")

import numpy as np

import concourse.bacc as bacc
import concourse.mybir as mybir
from concourse import bass_utils, dve_ops
from concourse.dve_ops import DveOp
from concourse.dve_spec import AluOp, Spec, Src0, Src1, lower, scan
from concourse.dve_uop import DveOpSpec
from concourse.tile import TileContext

P = 128
N = 32
N_CORES = 8
B = 131072

# tunables
VARIANT = "dve"  # "dve" | "cumsum"
F = 128  # batch slots per partition per mega-tile
NSUB = 2  # n per sub-tile (DMA block)
MEGA = 1  # mega-tiles per core; MEGA*P*F == B // N_CORES
HALVES = N // NSUB
B_CORE = B // N_CORES
assert MEGA * P * F == B_CORE


def _register_cumsum_mul():
    """Runtime-register the custom DVE op out[k] = sum_{u<=k} in0[u]*in1[u]."""
    if any(op.name == "CUMSUM_MUL" for op in dve_ops.OPS):
        return next(op for op in dve_ops.OPS if op.name == "CUMSUM_MUL")

    def _ref(in0, in1, s0, s1, imm2):
        prod = in0.astype(np.float32) * in1.astype(np.float32)
        flat = prod.reshape(prod.shape[0], -1)
        return np.cumsum(flat, axis=-1).reshape(prod.shape)

    spec = Spec(body=scan(AluOp.ADD, Src0 * Src1), reference=_ref)
    shas = {}
    for ver in ("v3", "v4"):
        tmp = DveOpSpec(name="CUMSUM_MUL", opcode=0, uops=lower(spec, ver=ver), rd1_en=True)
        shas[ver] = tmp.sha(ver)
    op = DveOp("CUMSUM_MUL", spec, subdim=False, uops_sha=shas)
    dve_ops.OPS.append(op)
    dve_ops.CUSTOM_DVE_SPECS[op.name] = op.spec
    dve_ops._SUB_OPCODE_FOR_NAME[op.name] = (
        dve_ops._CUSTOM_DVE_ROW_BASE + len(dve_ops.OPS) - 1
    )
    return op


CUMSUM_MUL = None  # registered lazily by build() for the cumsum variant


class Cfg:
    def __init__(self, F=F, NSUB=NSUB, MEGA=MEGA, variant=VARIANT):
        self.F = F
        self.NSUB = NSUB
        self.MEGA = MEGA
        self.HALVES = N // NSUB
        self.B_CORE = MEGA * P * F
        self.variant = variant


def _step_dve(nc, ppool, C, A, Bm, sh):
    eng = nc.vector
    F_ = sh[1]
    tmp = ppool.tile([P, F_ * 12], mybir.dt.float32, tag="tk")
    tv = tmp.rearrange("p (f i j) -> p f i j", f=F_, i=3)
    eng.tensor_mul(
        out=C,
        in0=A[:, :, :, 0:1].broadcast_to(sh),
        in1=Bm[:, :, 0:1, :].broadcast_to(sh),
    )
    eng.tensor_mul(
        out=tv,
        in0=A[:, :, :, 1:2].broadcast_to(sh),
        in1=Bm[:, :, 1:2, :].broadcast_to(sh),
    )
    eng.tensor_add(out=C, in0=C, in1=tv)
    eng.tensor_mul(
        out=tv,
        in0=A[:, :, :, 2:3].broadcast_to(sh),
        in1=Bm[:, :, 2:3, :].broadcast_to(sh),
    )
    eng.tensor_add(out=C, in0=C, in1=tv)
    eng.tensor_add(out=C[:, :, :, 3], in0=C[:, :, :, 3], in1=A[:, :, :, 3])


def _step_cumsum(nc, epool, sbuf_S, C, A, Bm, sh):
    """A/Bm/C: [P, F, 3, 4] views; sbuf_S: persistent [P, 36F+3] scan buffer
    with S[:,0] pre-zeroed."""
    F_ = sh[1]
    G = 36 * F_
    aexp = epool.tile([P, G], mybir.dt.float32, tag="aexp")
    bexp = epool.tile([P, G], mybir.dt.float32, tag="bexp")
    # stream position = f*36 + i*12 + j*3 + k
    for k in range(3):
        a_out = aexp.rearrange("p (f i j k2) -> p f i j k2", f=F_, i=3, j=4)[
            :, :, :, :, k
        ]
        b_out = bexp.rearrange("p (f i j k2) -> p f i j k2", f=F_, i=3, j=4)[
            :, :, :, :, k
        ]
        nc.scalar.copy(out=a_out, in_=A[:, :, :, k : k + 1].broadcast_to(sh))
        nc.scalar.copy(out=b_out, in_=Bm[:, :, k : k + 1, :].broadcast_to(sh))
    s_out = sbuf_S[:, 1 : 1 + G]
    nc.vector._custom_dve(CUMSUM_MUL, out=s_out, in0=aexp[:], in1=bexp[:])
    minu = sbuf_S[:, 3 : 3 + G].rearrange("p (f g k) -> p f g k", f=F_, g=12)[
        :, :, :, 0
    ]
    subt = sbuf_S[:, 0:G].rearrange("p (f g k) -> p f g k", f=F_, g=12)[:, :, :, 0]
    cflat = C.rearrange("p f i j -> p f (i j)")
    nc.vector.tensor_tensor(
        out=cflat, in0=minu, in1=subt, op=mybir.AluOpType.subtract
    )
    nc.vector.tensor_add(out=C[:, :, :, 3], in0=C[:, :, :, 3], in1=A[:, :, :, 3])


def build(cfg: Cfg):
    F, NSUB, MEGA, HALVES = cfg.F, cfg.NSUB, cfg.MEGA, cfg.HALVES
    BLK = F * NSUB * 12
    nc = bacc.Bacc("TRN2", target_bir_lowering=False, debug=False)
    x = nc.dram_tensor(
        "x", [MEGA * HALVES, P, BLK], mybir.dt.float32, kind="ExternalInput"
    )
    y = nc.dram_tensor(
        "y", [MEGA * HALVES, P, BLK], mybir.dt.float32, kind="ExternalOutput"
    )

    if cfg.variant == "cumsum":
        global CUMSUM_MUL
        CUMSUM_MUL = _register_cumsum_mul()

    with TileContext(nc) as tc:
        with (
            tc.tile_pool(name="xin", bufs=3) as xpool,
            tc.tile_pool(name="outp", bufs=3) as opool,
            tc.tile_pool(name="work", bufs=3) as wpool,
            tc.tile_pool(name="scanbuf", bufs=1) as spool,
        ):
            sbufs = []
            if cfg.variant == "cumsum":
                for t in range(MEGA):
                    st = spool.tile([P, 36 * F + 3], mybir.dt.float32, tag=f"s{t}")
                    nc.vector.memset(st[:, 0:1], 0.0)
                    sbufs.append(st)

            for t in range(MEGA):
                prev = None
                for h in range(HALVES):
                    xt = xpool.tile([P, BLK], mybir.dt.float32, tag="x")
                    nc.sync.dma_start(out=xt[:], in_=x.ap()[t * HALVES + h])
                    ot = opool.tile([P, BLK], mybir.dt.float32, tag="o")
                    xv = xt.rearrange("p (f n i j) -> p f n i j", f=F, n=NSUB, i=3)
                    ov = ot.rearrange("p (f n i j) -> p f n i j", f=F, n=NSUB, i=3)
                    for nl in range(NSUB):
                        if h == 0 and nl == 0:
                            nc.scalar.copy(out=ov[:, :, 0], in_=xv[:, :, 0])
                            continue
                        A = ov[:, :, nl - 1] if nl > 0 else prev[:, :, NSUB - 1]
                        Bm = xv[:, :, nl]
                        sh = [P, F, 3, 4]
                        if cfg.variant == "dve":
                            _step_dve(nc, wpool, ov[:, :, nl], A, Bm, sh)
                        else:
                            _step_cumsum(
                                nc, wpool, sbufs[t], ov[:, :, nl], A, Bm, sh
                            )
                    nc.sync.dma_start(out=y.ap()[t * HALVES + h], in_=ot[:])
                    prev = ov
    nc.compile()
    return nc


_NC_CACHE = []


def _get_nc():
    if not _NC_CACHE:
        _NC_CACHE.append(build(Cfg()))
    return _NC_CACHE[0]


def shard_input(x_full, cfg, n_cores=N_CORES):
    F, NSUB, MEGA, HALVES = cfg.F, cfg.NSUB, cfg.MEGA, cfg.HALVES
    out = []
    for c in range(n_cores):
        xc = x_full[c * cfg.B_CORE : (c + 1) * cfg.B_CORE].reshape(MEGA, P, F, N, 12)
        xc = xc.reshape(MEGA, P, F, HALVES, NSUB, 12)
        xc = np.ascontiguousarray(xc.transpose(0, 3, 1, 2, 4, 5))
        out.append(xc.reshape(MEGA * HALVES, P, F * NSUB * 12))
    return out


def unshard_output(ys, cfg):
    parts = []
    for yc in ys:
        a = yc.reshape(cfg.MEGA, cfg.HALVES, P, cfg.F, cfg.NSUB, 12)
        a = a.transpose(0, 2, 3, 1, 4, 5).reshape(cfg.B_CORE, N, 3, 4)
        parts.append(a)
    return np.concatenate(parts, axis=0)


def run(x, trace=False, trace_kwargs=None):
    """Returns (out [B,N,3,4], BassKernelResults)."""
    cfg = Cfg()
    x = np.asarray(x, dtype=np.float32).reshape(B, N, 12)
    nc = _get_nc()
    in_maps = [{"x": xc} for xc in shard_input(x, cfg)]
    res = bass_utils.run_bass_kernel_spmd(
        nc,
        in_maps,
        list(range(N_CORES)),
        trace=trace,
        **(trace_kwargs or {}),
    )
    out = unshard_output([r["y"] for r in res.results], cfg)
    return out.reshape(B, N, 3, 4), res


def kernel(x):
    return run(x)[0]



# revision 6
# speedup vs baseline: 1.1815x; 1.1815x over previous
"""SE(3) compose-scan Trainium2 kernel (nn_ComposeRt), fp16 scaled variant.

x [131072, 32, 3, 4] fp32 -> cumulative compose along axis 1:
out[b,0] = x[b,0]; out[b,n] = out[b,n-1] o x[b,n],
[rA|tA] o [rB|tB] = [rA@rB | tA + rA@tB].

Strategy:
- Pure data parallel over batch across 8 NeuronCores (b = c*16384 + p*128 + f).
- Host pre-scales x by s = 3^-0.5 and casts to fp16. With X' = s*X the
  recurrence C'_n = [R'@Xr' | s*t' + R'@xt'] gives C'_n = s^(n+1)*C_n with
  entries O(1..300) -- safely inside fp16 range, and fp16's 11-bit mantissa
  keeps the accumulated scan error ~2e-3 (gate 2e-2). Host unscales the
  fp16 output by s^-(n+1) in fp32. fp16 also halves DMA traffic and enables
  the DVE 2x_1P perf mode (2 elem/cycle) for the product/add instructions.
- Per core, 128 slots per partition split: F_D on the vector engine (two
  interleaved halves so the scalar engine's carry-expansion copy overlaps),
  F_G on gpsimd running the plain broadcast tensor-op sequence.
- DVE step (per half, DH slots): one fused multiply over all k
  (prod[k,f,i,j] = Aexp[f,k,i,j]*X[f,k,j], contiguous fp16 streams -> 2x),
  two plane adds (2x), one scalar_tensor_tensor for the translation column
  (C3 = s*A3 + C3). ACT rebuilds the expanded carry Aexp[f,k,i,j]=C[f,i,k]
  for the next step.
- n=0 is never computed on device; the host copies x[:,0] into the output.
"""

import sys

if "/opt/trn_rl_repo" not in sys.path:
    sys.path.insert(0, "/opt/trn_rl_repo")

import numpy as np

import concourse.bacc as bacc
import concourse.mybir as mybir
from concourse import bass_utils
from concourse.tile import TileContext

P = 128
N = 32
N_CORES = 8
B = 131072
SCALE = 3.0 ** -0.5

# tunables
F_D = 100  # DVE slots per partition (split into two interleaved halves)
F_G = 28   # gpsimd slots per partition
NSUB = 2   # n per DMA block
F = F_D + F_G
HALVES = N // NSUB
B_CORE = P * F
assert B_CORE * N_CORES == B
assert F_D % 2 == 0
DH = F_D // 2
FP16 = mybir.dt.float16


class Cfg:
    def __init__(self, f_d=F_D, f_g=F_G, nsub=NSUB):
        assert f_d % 2 == 0 and f_d + f_g == F
        self.F_D = f_d
        self.F_G = f_g
        self.DH = f_d // 2
        self.NSUB = nsub
        self.HALVES = N // nsub


def _a_col_view(c_flat, fx, k):
    """[P, fx, 12] carry view -> [P, fx, 3(i), 4(jrep)]: A[i,k] replicated
    along j. ISA limit: <=3 free dims per AP."""
    cv = c_flat.rearrange("p f (i j) -> p f i j", i=3)
    return cv[:, :, :, k].unsqueeze(3).broadcast_to([P, fx, 3, 4])


def _x_row_view(x_flat, fx, k):
    """[P, fx, 12] X view -> [P, fx, 3(irep), 4(j)]: X[k,j] replicated
    along i."""
    xv = x_flat.rearrange("p f (k j) -> p f k j", k=3)
    return xv[:, :, k].unsqueeze(2).broadcast_to([P, fx, 3, 4])


def build(cfg: Cfg):
    f_d, f_g, dh, nsub, halves = cfg.F_D, cfg.F_G, cfg.DH, cfg.NSUB, cfg.HALVES
    BLKD = f_d * nsub * 12
    BLKG = f_g * nsub * 12
    nc = bacc.Bacc("TRN2", target_bir_lowering=False, debug=False)
    xd = nc.dram_tensor("xd", [halves, P, BLKD], FP16, kind="ExternalInput")
    yd = nc.dram_tensor("yd", [halves, P, BLKD], FP16, kind="ExternalOutput")
    if f_g:
        xg = nc.dram_tensor("xg", [halves, P, BLKG], FP16, kind="ExternalInput")
        yg = nc.dram_tensor("yg", [halves, P, BLKG], FP16, kind="ExternalOutput")

    mult = mybir.AluOpType.mult
    add = mybir.AluOpType.add

    with TileContext(nc) as tc:
        with (
            tc.tile_pool(name="xin", bufs=3) as xpool,
            tc.tile_pool(name="outp", bufs=3) as opool,
            tc.tile_pool(name="prod", bufs=2) as prpool,
            tc.tile_pool(name="aexp", bufs=2) as aepool,
        ):
            aexp = [None, None]  # current Aexp tile per DVE half
            prev_d = None  # [P, f_d, 12] view of C_{n-1} (or X_0)
            prev_g = None
            if f_g:
                # fp16 constant tile holding SCALE for the gpsimd translation
                # step (Pool rejects scalar_tensor_tensor).
                sc = prpool.tile([P, 1], FP16, tag="sc")
                nc.vector.memset(sc[:], SCALE)
            for h in range(halves):
                xt = xpool.tile([P, BLKD], FP16, tag="xd")
                nc.sync.dma_start(out=xt[:], in_=xd.ap()[h])
                ot = opool.tile([P, BLKD], FP16, tag="od")
                xv = xt.rearrange("p (f n e) -> p f n e", f=f_d, n=nsub)
                ov = ot.rearrange("p (f n e) -> p f n e", f=f_d, n=nsub)
                if f_g:
                    xtg = xpool.tile([P, BLKG], FP16, tag="xg")
                    nc.sync.dma_start(out=xtg[:], in_=xg.ap()[h])
                    otg = opool.tile([P, BLKG], FP16, tag="og")
                    xvg = xtg.rearrange("p (f n e) -> p f n e", f=f_g, n=nsub)
                    ovg = otg.rearrange("p (f n e) -> p f n e", f=f_g, n=nsub)

                for nl in range(nsub):
                    n = h * nsub + nl
                    if n == 0:
                        # seed: A = X_0. Build Aexp for both halves on ACT.
                        for d in range(2):
                            fs = slice(d * dh, (d + 1) * dh)
                            ae = aepool.tile([P, dh * 36], FP16, tag=f"ae{d}")
                            aev = ae.rearrange("p (f k e) -> p f k e", f=dh, k=3)
                            for k in range(3):
                                nc.scalar.copy(
                                    out=aev[:, :, k].rearrange(
                                        "p f (i j) -> p f i j", i=3
                                    ),
                                    in_=_a_col_view(xv[:, fs, 0], dh, k),
                                )
                            aexp[d] = ae
                        prev_d = xv[:, :, 0]
                        if f_g:
                            prev_g = xvg[:, :, 0]
                        continue

                    # ---------------- DVE halves ----------------
                    for d in range(2):
                        fs = slice(d * dh, (d + 1) * dh)
                        pr = prpool.tile([P, 3 * dh * 12], FP16, tag=f"pr{d}")
                        pk = pr.rearrange("p (k f e) -> p k f e", k=3, f=dh)
                        aev = aexp[d].rearrange("p (f k e) -> p f k e", f=dh, k=3)
                        xn = xv[:, fs, nl]
                        for k in range(3):
                            nc.vector.tensor_mul(
                                out=pk[:, k].rearrange("p f (i j) -> p f i j", i=3),
                                in0=aev[:, :, k].rearrange(
                                    "p f (i j) -> p f i j", i=3
                                ),
                                in1=_x_row_view(xn, dh, k),
                            )
                        planes = pr.rearrange("p (k e) -> p k e", k=3)
                        nc.vector.tensor_add(
                            out=planes[:, 0], in0=planes[:, 0], in1=planes[:, 1]
                        )
                        cflat = ov[:, fs, nl]  # [P, dh, 12]
                        nc.vector.tensor_add(out=cflat, in0=pk[:, 0], in1=pk[:, 2])
                        cv = cflat.rearrange("p f (i j) -> p f i j", i=3)
                        pv = prev_d[:, fs].rearrange("p f (i j) -> p f i j", i=3)
                        nc.vector.scalar_tensor_tensor(
                            out=cv[:, :, :, 3],
                            in0=pv[:, :, :, 3],
                            scalar=SCALE,
                            in1=cv[:, :, :, 3],
                            op0=mult,
                            op1=add,
                        )
                        if n < N - 1:
                            ae = aepool.tile([P, dh * 36], FP16, tag=f"ae{d}")
                            aev = ae.rearrange("p (f k e) -> p f k e", f=dh, k=3)
                            for k in range(3):
                                nc.scalar.copy(
                                    out=aev[:, :, k].rearrange(
                                        "p f (i j) -> p f i j", i=3
                                    ),
                                    in_=_a_col_view(cflat, dh, k),
                                )
                            aexp[d] = ae

                    # ---------------- gpsimd slots ----------------
                    if f_g:
                        prg = prpool.tile([P, 3 * f_g * 12], FP16, tag="prg")
                        pkg = prg.rearrange("p (k f e) -> p k f e", k=3, f=f_g)
                        xng = xvg[:, :, nl]
                        for k in range(3):
                            nc.gpsimd.tensor_mul(
                                out=pkg[:, k].rearrange("p f (i j) -> p f i j", i=3),
                                in0=_a_col_view(prev_g, f_g, k),
                                in1=_x_row_view(xng, f_g, k),
                            )
                        planesg = prg.rearrange("p (k e) -> p k e", k=3)
                        nc.gpsimd.tensor_add(
                            out=planesg[:, 0], in0=planesg[:, 0], in1=planesg[:, 1]
                        )
                        cflatg = ovg[:, :, nl]
                        nc.gpsimd.tensor_add(out=cflatg, in0=pkg[:, 0], in1=pkg[:, 2])
                        cvg = cflatg.rearrange("p f (i j) -> p f i j", i=3)
                        pvg = prev_g.rearrange("p f (i j) -> p f i j", i=3)
                        # C3 = s*A3 + C3 via mul+add (reuse plane1's j=3 col
                        # as scratch; plane1 was already consumed by add1).
                        scr3 = pkg[:, 1].rearrange("p f (i j) -> p f i j", i=3)[
                            :, :, :, 3
                        ]
                        nc.gpsimd.tensor_mul(
                            out=scr3,
                            in0=pvg[:, :, :, 3],
                            in1=sc.unsqueeze(2).broadcast_to([P, f_g, 3]),
                        )
                        nc.gpsimd.tensor_add(
                            out=cvg[:, :, :, 3], in0=scr3, in1=cvg[:, :, :, 3]
                        )

                    prev_d = ov[:, :, nl]
                    if f_g:
                        prev_g = ovg[:, :, nl]

                nc.sync.dma_start(out=yd.ap()[h], in_=ot[:])
                if f_g:
                    nc.sync.dma_start(out=yg.ap()[h], in_=otg[:])
    nc.compile()
    return nc


_NC_CACHE = []


def _get_nc():
    if not _NC_CACHE:
        _NC_CACHE.append(build(Cfg()))
    return _NC_CACHE[0]


def _to_blocks(a, fx, cfg):
    """[P, fx, N, 12] -> [HALVES, P, fx*NSUB*12]"""
    h, ns = cfg.HALVES, cfg.NSUB
    b = a.reshape(P, fx, h, ns, 12).transpose(2, 0, 1, 3, 4)
    return np.ascontiguousarray(b).reshape(h, P, fx * ns * 12)


def _from_blocks(a, fx, cfg):
    """[HALVES, P, fx*NSUB*12] -> [P, fx, N, 12]"""
    h, ns = cfg.HALVES, cfg.NSUB
    b = a.reshape(h, P, fx, ns, 12).transpose(1, 2, 0, 3, 4)
    return b.reshape(P, fx, N, 12)


def shard_input(x, cfg):
    xs = (x.reshape(B, N, 12) * SCALE).astype(np.float16)
    maps = []
    for c in range(N_CORES):
        xc = xs[c * B_CORE : (c + 1) * B_CORE].reshape(P, cfg.F_D + cfg.F_G, N, 12)
        m = {"xd": _to_blocks(xc[:, : cfg.F_D], cfg.F_D, cfg)}
        if cfg.F_G:
            m["xg"] = _to_blocks(xc[:, cfg.F_D :], cfg.F_G, cfg)
        maps.append(m)
    return maps


def unshard_output(results, x, cfg):
    inv = (1.0 / SCALE) ** (np.arange(N, dtype=np.float64) + 1)
    inv = inv.astype(np.float32)[None, :, None]
    parts = []
    for r in results:
        pd = _from_blocks(r["yd"], cfg.F_D, cfg)
        if cfg.F_G:
            pg = _from_blocks(r["yg"], cfg.F_G, cfg)
            pc = np.concatenate([pd, pg], axis=1)
        else:
            pc = pd
        parts.append(pc.reshape(B_CORE, N, 12))
    out = np.concatenate(parts, axis=0).astype(np.float32) * inv
    out[:, 0] = x.reshape(B, N, 12)[:, 0]
    return out.reshape(B, N, 3, 4)


def run(x, trace=False, trace_kwargs=None):
    cfg = Cfg()
    x = np.asarray(x, dtype=np.float32)
    nc = _get_nc()
    in_maps = shard_input(x, cfg)
    res = bass_utils.run_bass_kernel_spmd(
        nc,
        in_maps,
        list(range(N_CORES)),
        trace=trace,
        **(trace_kwargs or {}),
    )
    out = unshard_output(res.results, x, cfg)
    return out, res


def kernel(x):
    return run(x)[0]
